# revision 1
# baseline (speedup 1.0000x reference)
"""Bottom-Up HTMM upward pass on 8 Trainium2 NeuronCores — v2.

Problem: complete 8-ary forest (2 trees x 299593 nodes, depth 6), C=8 hidden
states, 32 symbols, 16 generative models. Output: per-tree log-likelihood
(2, 16).

Sharding: core = (tree, quarter-of-tree); each core runs the upward pass over
its quarter (2 complete depth-1 subtrees): 65536 leaves -> 8192 -> 1024 -> 128
level-3 betas. Host finishes levels 2, 1 and the root step in f64 (18 nodes
per tree — pure latency on device, trivial on host).

v2 restructurings vs the v1 baseline:
  - Position-major node ordering per level (host-side permutation of xs/Bx):
    the W-matmul rhs for child-position l is the CONTIGUOUS slice
    child[:, l*U:(l+1)*U] — strided-rhs matmuls cost ~3.7x in the PE.
  - B[:, x_u] for all interior nodes is gathered on the HOST and DMA'd in as
    one bf16 table (Bxall). Kills the per-chunk one-hot+matmul+PSUM->SBUF-copy
    chain for parent symbols.
  - 1/nu broadcast 16->128 partitions: level 0 via DRAM-bounce DMA (overlaps
    across chunk groups, scalar HWDGE ring); level 1/2 via E16 matmul into
    PSUM (no DMA latency on the serial tail).
  - Level-0 stages are software-pipelined (emission order A0 A1 B0 A2 C0 A3
    B1 C1) so FIFO heads never block on cross-engine latency chains.
  - The 4 packed selt (nu) matmuls per group sit at 32-partition col offsets
    and are emitted back-to-back -> run concurrently in the PE array.
  - bl = tb(PSUM) * Bx multiply is column-split between DVE and Pool/GpSimd.
  - Leaf ll histogram counts and interior ll partials are shipped out raw
    (tiny) and folded on the host in f64.
Partition packing everywhere: p = i*16 + g  (i = hidden state, g = generator).
"""
import sys

import numpy as np

if '/opt/trn_rl_repo' not in sys.path:
    sys.path.insert(0, '/opt/trn_rl_repo')

import ml_dtypes

BF16 = ml_dtypes.bfloat16

K, DEPTH, NTREE, C, MSYM, NGEN = 8, 6, 2, 8, 32, 16
STARTS = [(K ** d - 1) // (K - 1) for d in range(DEPTH + 2)]
NT = STARTS[DEPTH + 1]          # 299593 nodes per tree
CG = C * NGEN                   # 128
NQ = 4                          # quarters per tree
LEAVES_Q = (K ** DEPTH) // NQ   # 65536 leaves per core
XP_PAD = 9376                   # interior-node Bx slots per core (padded)
# (parents U, chunks, Bx offset); chunk width N = U // chunks
LEVELS = [
    (8192, 8, 0),
    (1024, 1, 8192),
    (128, 1, 9216),
]
N_LL_SLOTS = 4   # 2 level-0 groups + levels 4, 3
USIZES = [8192, 1024, 128, 16, 2]   # per-core nodes at levels 5,4,3,2,1


def _sigmas():
    """Position-major orderings per level: sigma[d][n_natural] = stored col."""
    sig = {1: np.arange(2)}
    for i, d in enumerate([2, 3, 4, 5]):
        U = USIZES[3 - i]
        n = np.arange(U)
        sig[d] = (n % K) * (U // K) + sig[d - 1][n // K]
    return sig


def _softmax64(x, axis):
    x = np.asarray(x, np.float64)
    e = np.exp(x - x.max(axis=axis, keepdims=True))
    return e / e.sum(axis=axis, keepdims=True)


def _build_tables(A, B, Pi, SP):
    """Small O(params) tables, f64 on host."""
    smA = _softmax64(A, 0)            # (C,C,K,G) over parent state i
    smB = _softmax64(B, 1)            # (C,M,G) over symbols
    smPi = _softmax64(Pi, 0)          # (C,K,G)
    smSP = _softmax64(SP, 0)          # (K,G)
    Mmat = smSP[:, None, None, :] * np.transpose(smA, (2, 0, 1, 3))  # [l,i,j,g]
    pb = smPi[:, :, None, :] * smB[:, None, :, :]     # (j, l, s, g)
    nuL = pb.sum(0)                                    # (l, s, g)
    betaLeaf = pb / nuL[None]
    llLeaf = np.log(nuL)                               # (l, s, g)
    T6 = np.einsum('lijg,jlsg->lsig', Mmat, betaLeaf)  # (l,s,i,g)
    T6f = T6.reshape(K * MSYM, CG)                     # rows (l,s), cols (i,g)
    Wl = np.zeros((K, CG, CG))
    ii = np.arange(C)
    for l in range(K):
        for g in range(NGEN):
            Wl[l, ii[:, None] * NGEN + g, ii[None, :] * NGEN + g] = Mmat[l, :, :, g].T
    p = np.arange(CG)
    sel = (p[:, None] % NGEN == np.arange(NGEN)[None, :]).astype(np.float64)
    E16 = sel.T.copy()                                 # [16, 128]
    E16x4 = np.concatenate(
        [np.vstack([E16, np.zeros((16, CG))]) for _ in range(4)], axis=0)
    lo = np.concatenate([T6f[l * 32: l * 32 + 16] for l in range(K)], axis=0)
    hi = np.concatenate([T6f[l * 32 + 16: (l + 1) * 32] for l in range(K)], axis=0)
    return {
        'T6lo': lo.astype(BF16),
        'T6hi': hi.astype(BF16),
        'Wt': np.concatenate([Wl[l] for l in range(K)], axis=1).astype(BF16),  # [128, 1024]
        'selt': sel.astype(BF16),                     # [128, 16]
        'selt32': np.concatenate([sel, sel], axis=1).astype(BF16),  # [128, 32]
        'E16t': E16.astype(BF16),                     # [16, 128]
        'E16x4': E16x4.astype(BF16),                  # [128, 128]
        'svlo': (np.arange(128) % 16).astype(np.float32).reshape(128, 1),
        'svhi': (np.arange(128) % 16 + 16).astype(np.float32).reshape(128, 1),
    }, Mmat, smB, llLeaf


def _build_bass():
    import concourse.bass as bass
    import concourse.bacc as bacc
    import concourse.mybir as mybir
    from concourse import tile

    f32 = mybir.dt.float32
    bf16 = mybir.dt.bfloat16
    u8 = mybir.dt.uint8
    Alu = mybir.AluOpType
    Act = mybir.ActivationFunctionType

    nc = bacc.Bacc(None, target_bir_lowering=False)

    xs_d = nc.dram_tensor('xs', [K, LEAVES_Q // K], u8, kind='ExternalInput')
    bx_d = nc.dram_tensor('bxall', [128, XP_PAD], bf16, kind='ExternalInput')
    tab_specs = [
        ('svlo', [128, 1], f32), ('svhi', [128, 1], f32),
        ('T6lo', [128, 128], bf16), ('T6hi', [128, 128], bf16),
        ('selt', [128, 16], bf16), ('selt32', [128, 32], bf16),
        ('E16t', [16, 128], bf16),
        ('E16x4', [128, 128], bf16),
        ('Wt', [128, 1024], bf16),
    ]
    tab_d = {n: nc.dram_tensor(n, s, d, kind='ExternalInput') for n, s, d in tab_specs}
    beta3_d = nc.dram_tensor('beta3', [128, 128], f32, kind='ExternalOutput')
    llparts_d = nc.dram_tensor('llparts', [128, N_LL_SLOTS], f32, kind='ExternalOutput')
    cnts_d = nc.dram_tensor('cnts', [128, 12], f32, kind='ExternalOutput')
    # DRAM bounce buffer for the 16->128 partition broadcast of 1/nu, level 0
    # (SBUF sources cannot have stride-0 partition dims; DRAM sources can)
    rcp0_d = nc.dram_tensor('rcp0sc', [16, 4096], bf16, kind='Internal')

    GRP_OH = 2      # chunks per one-hot batch at level 0 (NW = 2048)
    GRP_NU = 4      # chunks per packed-nu group at level 0
    DVE_COLS = 640  # bl-multiply column split: [0:DVE_COLS] DVE, rest Pool

    with tile.TileContext(nc) as tc:
        with (
            tc.tile_pool(name='const', bufs=1) as constp,
            tc.tile_pool(name='beta', bufs=1) as betap,
            tc.tile_pool(name='oh', bufs=4) as ohp,
            tc.tile_pool(name='xsb', bufs=1) as xsbp,
            tc.tile_pool(name='bl', bufs=6) as blp,
            tc.tile_pool(name='tbc', bufs=4) as tbcp,
            tc.tile_pool(name='ln', bufs=2) as lnp,
            tc.tile_pool(name='rcp', bufs=2) as rcpp,
            tc.tile_pool(name='rb', bufs=4) as rbp,
            tc.tile_pool(name='acc', bufs=1) as accp,
            tc.tile_pool(name='ps_tb', bufs=2, space='PSUM') as ps_tb,
            tc.tile_pool(name='ps_nu', bufs=1, space='PSUM') as ps_nu,
            tc.tile_pool(name='ps_t1', bufs=1, space='PSUM') as ps_t1,
        ):
            tab = {}
            for n, s, d in tab_specs:
                tab[n] = constp.tile(s, d, name=n, tag=n)
            bxall = constp.tile([128, XP_PAD], bf16, name='bxall', tag='bxall')
            xsb = xsbp.tile([128, LEAVES_Q // K], u8, name='xsb', tag='xsb')

            def load_tab(names):
                for n in names:
                    nc.sync.dma_start(tab[n][:], tab_d[n][:])

            def load_xsb(g2):
                # one xs tile [128 = 8 l-rows x 16 reps, 8192]: 16x replication
                NW = 2048
                src_ap = bass.AP(
                    xs_d[:].tensor, g2 * NW,
                    [[LEAVES_Q // K, 8], [0, 16], [1, NW]])
                nc.sync.dma_start(xsb[:, g2 * NW:(g2 + 1) * NW], src_ap)

            # demand-ordered input loads: first group's one-hot sources and
            # tables first, bulk Bx last (shares the DMA pipe)
            load_tab(['svlo', 'svhi'])
            load_xsb(0)
            load_tab(['T6lo', 'T6hi', 'selt32'])
            for g2 in (1, 2, 3):
                load_xsb(g2)

            beta_bufs = [
                betap.tile([128, 8192], bf16, name='b5', tag='b5'),
                betap.tile([128, 1024], bf16, name='b4', tag='b4'),
                betap.tile([128, 128], f32, name='b3', tag='b3'),
            ]
            llparts = accp.tile([128, N_LL_SLOTS], f32, name='llparts', tag='llparts')
            cnts = accp.tile([128, 12], f32, name='cnts', tag='cnts')
            nc.vector.memset(llparts[:], 0.0)


            def mm512(out_ap, lhsT, rhs_ap, ncols, start, stop, tile_position=None):
                for s0 in range(0, ncols, 512):
                    s1 = min(s0 + 512, ncols)
                    kw = {}
                    if tile_position is not None:
                        kw['tile_position'] = tile_position
                    nc.tensor.matmul(out_ap[:, s0:s1], lhsT, rhs_ap[:, s0:s1],
                                     start=start, stop=stop, **kw)

            # ---- level 0: software-pipelined stages over 4 pairs / 2 groups
            N0 = 1024
            state = {'pend': {0: [], 1: []}, 'nu': {}}

            def stageA(pair):
                c2 = pair * 2
                g2 = c2 // GRP_OH
                grp = c2 // GRP_NU
                NW = N0 * GRP_OH
                # deferred bulk loads: scalar ring, just-in-time
                nc.scalar.dma_start(bxall[:, pair * 2048:(pair + 1) * 2048],
                                    bx_d[:, pair * 2048:(pair + 1) * 2048])
                if pair == 1:
                    load_tab(['selt'])
                if pair == 2:
                    load_tab(['Wt', 'E16x4', 'E16t'])
                if pair == 3:
                    nc.scalar.dma_start(bxall[:, 8192:XP_PAD],
                                        bx_d[:, 8192:XP_PAD])
                ohA = ohp.tile([128, NW], bf16, name='ohA', tag='ohA')
                ohB = ohp.tile([128, NW], bf16, name='ohB', tag='ohB')
                pieces = ((0, 512), (512, 1024), (1024, 2048)) if g2 == 0 else ((0, NW),)
                for svi, (sv, oh_t) in enumerate((('svlo', ohA), ('svhi', ohB))):
                    base = 6 * svi + (g2 + 2 if g2 > 0 else 0)
                    for pi, (p0, p1) in enumerate(pieces):
                        nc.vector.tensor_scalar(
                            oh_t[:, p0:p1], xsb[:, g2 * NW + p0:g2 * NW + p1],
                            tab[sv][:], None,
                            Alu.is_equal, Alu.add,
                            accum_out=cnts[:, base + pi:base + pi + 1])
                if c2 % GRP_NU == 0:
                    nu_ps = ps_nu.tile([128, N0], f32, name='nu4', tag='nu4')
                    state['nu'][grp] = nu_ps
                tbs = [ps_tb.tile([128, N0], f32, name='tb', tag='tb')
                       for _ in range(2)]
                for k in range(2):
                    mm512(tbs[k][:], tab['T6lo'][:], ohA[:, k * N0:(k + 1) * N0],
                          N0, True, False)
                for k in range(2):
                    mm512(tbs[k][:], tab['T6hi'][:], ohB[:, k * N0:(k + 1) * N0],
                          N0, False, True)
                for k in range(2):
                    c = c2 + k
                    tbc = tbcp.tile([128, N0], bf16, name='tbc', tag='tbc')
                    nc.scalar.copy(tbc[:], tbs[k][:])
                    bl_t = blp.tile([128, N0], bf16, name='bl', tag='bl')
                    bxs = bxall[:, c * N0: (c + 1) * N0]
                    nc.vector.tensor_mul(bl_t[:, 0:DVE_COLS],
                                         tbc[:, 0:DVE_COLS], bxs[:, 0:DVE_COLS])
                    nc.gpsimd.tensor_mul(bl_t[:, DVE_COLS:N0],
                                         tbc[:, DVE_COLS:N0], bxs[:, DVE_COLS:N0])
                    # nu contribution immediately (col-tiled into packed psum)
                    nu_ps = state['nu'][grp]
                    poff = 32 * (c % GRP_NU)
                    mm512(nu_ps[poff:poff + 32, :], tab['selt32'][:],
                          bl_t[:], N0, True, True, tile_position=(0, poff))
                    state['pend'][grp].append((bl_t, c))

            def stageB(grp, use_dma):
                nu_ps = state['nu'][grp]
                rcp_t = rcpp.tile([128, N0], bf16, name='rcp', tag='rcp')
                with nc.allow_low_precision(reason='bf16 recip; ll tolerance is 2e-2'):
                    nc.vector.reciprocal(rcp_t[:], nu_ps[:])
                ln_t = lnp.tile([128, N0], f32, name='ln', tag='ln')
                nc.scalar.activation(ln_t[:], nu_ps[:], Act.Ln,
                                     accum_out=llparts[:, grp:grp + 1])
                if use_dma:
                    for qi, (_, cp) in enumerate(state['pend'][grp]):
                        poff = 32 * (cp % GRP_NU)
                        ring = nc.sync if qi % 2 == 0 else nc.scalar
                        ring.dma_start(rcp0_d[:, cp * N0:(cp + 1) * N0],
                                       rcp_t[poff:poff + 16, :])
                    for qi, (_, cp) in enumerate(state['pend'][grp]):
                        rb_t = rbp.tile([128, N0], bf16, name='rb', tag='rb')
                        rb_src = bass.AP(rcp0_d[:].tensor, cp * N0,
                                         [[0, 8], [4096, 16], [1, N0]])
                        ring = nc.sync if qi % 2 == 0 else nc.scalar
                        ring.dma_start(rb_t[:], rb_src)
                        state.setdefault('rb', {}).setdefault(grp, []).append(rb_t)
                else:
                    # E16 broadcast into PSUM — no DMA latency on the tail
                    for _, cp in state['pend'][grp]:
                        poff = 32 * (cp % GRP_NU)
                        rb_ps = ps_tb.tile([128, N0], f32, name='rbp', tag='tb')
                        mm512(rb_ps, tab['E16x4'][poff:poff + 16, :],
                              rcp_t[poff:poff + 16, :], N0, True, True,
                              tile_position=(poff, 0))
                        state.setdefault('rb', {}).setdefault(grp, []).append(rb_ps)

            # level-4 tb accumulates inside stageC: W-matmul for position l
            # needs only level-0 chunk l's beta (position-major layout)
            tb1_ps = ps_t1.tile([128, 1024], f32, name='tb1', tag='tb1')

            def stageC(grp, split=False):
                b5 = beta_bufs[0]
                for (bl_p, cp), rb_t in zip(state['pend'][grp], state['rb'][grp]):
                    if split:
                        nc.vector.tensor_mul(b5[:, cp * N0:cp * N0 + DVE_COLS],
                                             bl_p[:, 0:DVE_COLS], rb_t[:, 0:DVE_COLS])
                        nc.gpsimd.tensor_mul(b5[:, cp * N0 + DVE_COLS:(cp + 1) * N0],
                                             bl_p[:, DVE_COLS:N0], rb_t[:, DVE_COLS:N0])
                    else:
                        nc.vector.tensor_mul(b5[:, cp * N0:(cp + 1) * N0], bl_p[:], rb_t[:])
                    mm512(tb1_ps[:], tab['Wt'][:, 128 * cp:128 * (cp + 1)],
                          b5[:, cp * N0:(cp + 1) * N0],
                          N0, cp == 0, cp == K - 1)
                state['pend'][grp] = []

            stageA(0); stageA(1); stageB(0, True); stageA(2); stageA(3)
            stageC(0, split=True); stageB(1, False); stageC(1, split=False)
            nc.sync.dma_start(cnts_d[:], cnts[:])

            # ---- levels 1..2 (level-4 and level-3 nodes) ----
            slot = 2
            for lev in (1, 2):
                U, _, bxo = LEVELS[lev]
                N = U
                child = beta_bufs[lev - 1]
                out_beta = beta_bufs[lev]
                if lev == 1:
                    tb_ps = tb1_ps
                else:
                    tb_ps = state['tb2']   # accumulated piecewise during lev 1
                bl_t = blp.tile([128, N], bf16, name='bl', tag='bl')
                bxs = bxall[:, bxo: bxo + N]
                nc.vector.tensor_mul(bl_t[:, 0:N], tb_ps[:, 0:N], bxs[:, 0:N])
                nu_ps = ps_nu.tile([16, N], f32, name='nu1', tag='nu4')
                mm512(nu_ps[:], tab['selt'][:], bl_t[:], N, True, True)
                rcp_t = rcpp.tile([16, N], bf16, name='rcp', tag='rcp')
                with nc.allow_low_precision(reason='bf16 recip; ll tolerance is 2e-2'):
                    nc.vector.reciprocal(rcp_t[:], nu_ps[:])
                ln_t = lnp.tile([16, N], f32, name='ln', tag='ln')
                nc.scalar.activation(ln_t[:], nu_ps[:], Act.Ln,
                                     accum_out=llparts[0:16, slot:slot + 1])
                # 16->128 broadcast on the PE (tail latency beats DMA here)
                rb_ps = ps_tb.tile([128, N], f32, name='rbp', tag='tb')
                mm512(rb_ps[:], tab['E16t'][:], rcp_t[:], N, True, True)
                if lev == 1:
                    # piecewise beta so level-3 W-matmuls start per 128-col
                    # block (position-major: block l feeds W-matmul l)
                    tb2_ps = ps_t1.tile([128, 128], f32, name='tb2', tag='tb1')
                    state['tb2'] = tb2_ps
                    for piece in range(4):
                        sl = slice(piece * 256, (piece + 1) * 256)
                        nc.vector.tensor_mul(out_beta[:, sl], bl_t[:, sl], rb_ps[:, sl])
                        for l in (2 * piece, 2 * piece + 1):
                            nc.tensor.matmul(
                                tb2_ps[:], tab['Wt'][:, 128 * l:128 * (l + 1)],
                                out_beta[:, l * 128:(l + 1) * 128],
                                start=(l == 0), stop=(l == K - 1))
                else:
                    nc.vector.tensor_mul(out_beta[:], bl_t[:], rb_ps[:, 0:N])
                slot += 1

            nc.sync.dma_start(llparts_d[:], llparts[:])
            nc.sync.dma_start(beta3_d[:], beta_bufs[2][:])
    if not nc.is_finalized():
        nc.finalize()
    return nc


_BASS_CACHE = {}


def _get_bass():
    if 'nc' not in _BASS_CACHE:
        _BASS_CACHE['nc'] = _build_bass()
    return _BASS_CACHE['nc']


def kernel(**inputs):
    from concourse.bass_utils import run_bass_kernel_spmd

    A = np.asarray(inputs['A']); B = np.asarray(inputs['B'])
    Pi = np.asarray(inputs['Pi']); SP = np.asarray(inputs['SP'])
    x = np.asarray(inputs['x'])

    tables, Mmat, smB, llLeaf = _build_tables(A, B, Pi, SP)
    BT = np.transpose(smB, (1, 0, 2)).reshape(MSYM, CG)  # [sym, (i,g)] f64
    sig = _sigmas()

    in_maps = []
    for t in range(NTREE):
        base = t * NT
        for q in range(NQ):
            s6 = base + STARTS[6] + q * LEAVES_Q
            xs = x[s6: s6 + LEAVES_Q].astype(np.uint8)
            # xs_t[l, sigma5(p)] = symbol of child l of level-5 parent p
            xs_t = np.empty((K, LEAVES_Q // K), np.uint8)
            xs_t[:, sig[5]] = xs.reshape(LEAVES_Q // K, K).T
            xs_t = np.ascontiguousarray(xs_t)
            # interior symbols, position-major per level (levels 5, 4, 3)
            xp = np.zeros(XP_PAD, np.int64)
            off = 0
            for d in range(5, 2, -1):
                n_d = K ** d
                s = base + STARTS[d] + q * (n_d // NQ)
                lev_x = x[s: s + n_d // NQ]
                perm = np.empty(n_d // NQ, np.int64)
                perm[sig[d]] = lev_x
                xp[off: off + n_d // NQ] = perm
                off += n_d // NQ
            bxall = np.ascontiguousarray(BT[xp].T).astype(BF16)  # [128, XP_PAD]
            m = {'xs': xs_t, 'bxall': bxall}
            m.update(tables)
            in_maps.append(m)

    nc = _get_bass()
    global _LAST_IN_MAPS
    _LAST_IN_MAPS = in_maps
    res = run_bass_kernel_spmd(nc, in_maps, core_ids=list(range(8)))
    results = res.results

    llLf = llLeaf.reshape(K * MSYM, NGEN)   # rows (l, s), f64
    out = np.zeros((NTREE, NGEN), np.float64)
    inv3 = np.argsort(sig[3])               # stored col -> natural node
    for t in range(NTREE):
        base = t * NT
        beta3 = np.zeros((512, C, NGEN), np.float64)   # level-3, natural order
        for q in range(NQ):
            r = results[t * NQ + q]
            lp = np.asarray(r['llparts'], np.float64)
            cn = np.asarray(r['cnts'], np.float64)
            # level-0 slots 0,1: rows 32c+g hold 4-chunk-group partials
            p = np.arange(128)
            g = p % 32
            lvl0 = lp[:, 0:2][g < 16]
            gv = g[g < 16]
            for gg in range(NGEN):
                out[t, gg] += lvl0[gv == gg].sum()
            # level-4/3 slots 2,3: rows 0..15
            out[t, :] += lp[0:16, 2:4].sum(axis=1)
            # leaf histogram: cnts rows (l4, s); slots 0-3 = l 0..3, 4-7 = l 4..7
            cA = cn[:, 0:6].sum(axis=1)          # rows (l, s<16) = l*16+s
            cB = cn[:, 6:12].sum(axis=1)         # rows (l, s>=16) = l*16+s-16
            hist = np.empty(K * MSYM)
            for l in range(K):
                hist[l * 32: l * 32 + 16] = cA[l * 16:(l + 1) * 16]
                hist[l * 32 + 16: (l + 1) * 32] = cB[l * 16:(l + 1) * 16]
            out[t, :] += hist @ llLf
            # beta3 dram [128, 128]: stored cols sigma3, p = i*16+g
            b3 = np.asarray(r['beta3'], np.float64).T[inv3]      # [128 nodes, 128]
            beta3[q * 128:(q + 1) * 128] = b3.reshape(128, C, NGEN)
        # host: levels 2, 1, root in f64
        mcur = beta3
        for d in (2, 1):
            n_d = K ** d
            xs_lev = x[base + STARTS[d]: base + STARTS[d + 1]]
            bch = mcur.reshape(n_d, K, C, NGEN)
            tb = np.einsum('uljg,lijg->uig', bch, Mmat)
            bl = tb * np.transpose(smB[:, xs_lev], (1, 0, 2))
            nu = bl.sum(1)
            mcur = bl / nu[:, None]
            out[t] += np.log(nu).sum(0)
        tb = np.einsum('ljg,lijg->ig', mcur.reshape(K, C, NGEN), Mmat)
        bl = tb * smB[:, x[base]]
        out[t] += np.log(bl.sum(0))
    return out.astype(np.float32)

